# revision 8
# baseline (speedup 1.0000x reference)
"""Rotated RoIAlign (7x7, bilinear, zero-padding) for Trainium2, 8 NeuronCores.

Data-parallel sharding: 1024 boxes (2 images x 512) split into 8 groups of
128 boxes; core k handles image k//4, box slice (k%4)*128:(k%4+1)*128.

Strategy per core:
  - feature map supplied x-major channels-last, split by x-column parity:
    E[xc/2*H + y, :] = fm[:, y, xc] for even xc, O likewise for odd xc
    (30400 rows of 256 f32 each -> row indices fit the gather's int16).
  - box -> affine sample coords computed on-device (ACT Sin + DVE).
  - per sample point the bilinear footprint is columns {x0, x0+1} (one
    even, one odd) x rows {y0, y0+1}.  Two 2 KB dma_gather elements per
    point (one from E, one from O, elem = 2 consecutive y rows) fetch all
    4 corners; gathers are spread over 2 SWDGE queues.
  - weighted 4-slot sum via DVE scalar_tensor_tensor (per-partition scalar
    weights), output stored as [box, point, 256]; host transposes to
    [box, 256, 7, 7].
"""

import sys

for _p in ("/opt/trn_rl_repo", "/opt/pypackages"):
    if _p not in sys.path:
        sys.path.insert(0, _p)

import math

import numpy as np

B, C, H, W = 2, 256, 200, 304
N = 512            # boxes per image
OUT_H = OUT_W = 7
NPTS = OUT_H * OUT_W          # 49
P = 128                       # boxes per core
N_CORES = 8
GROUP = 7                     # points per gather call
NROWS = (W // 2) * H          # 30400 rows in each of E / O

_PI = math.pi
_TWO_PI = 2.0 * math.pi
_PI_CLAMP = 3.141592          # strictly inside f32(pi); ACT Sin domain guard
_MAGIC = float(3 * 2 ** 22)   # round-to-nearest-int magic for |x| < 2^22

_compiled = None


def _build_program():
    from concourse import bacc, bass, mybir
    import concourse.tile as tile

    f32 = mybir.dt.float32
    f16 = mybir.dt.float16
    i16 = mybir.dt.int16
    Alu = mybir.AluOpType
    Act = mybir.ActivationFunctionType

    nc = bacc.Bacc("TRN2", target_bir_lowering=False, debug=False,
                   num_devices=N_CORES, num_swdge_queues=2)

    fme = nc.dram_tensor("fme", [NROWS, C], f16, kind="ExternalInput")
    fmo = nc.dram_tensor("fmo", [NROWS, C], f16, kind="ExternalInput")
    boxes_d = nc.dram_tensor("boxes", [P, 5], f32, kind="ExternalInput")
    xs_d = nc.dram_tensor("xs", [P, NPTS], f32, kind="ExternalInput")
    ys_d = nc.dram_tensor("ys", [P, NPTS], f32, kind="ExternalInput")
    out_d = nc.dram_tensor("out", [P, NPTS, C], f32, kind="ExternalOutput")
    stge = nc.dram_tensor("stge", [P, NPTS], i16)     # idx staging (internal)
    stgo = nc.dram_tensor("stgo", [P, NPTS], i16)

    # overlapping-window view: unit stride = one row (1 KB), element = 2 rows
    fme_v = bass.AP(fme.ap().tensor, 0, [[C, NROWS - 1], [1, 2 * C]])
    fmo_v = bass.AP(fmo.ap().tensor, 0, [[C, NROWS - 1], [1, 2 * C]])

    with tile.TileContext(nc) as tc:
        with (
            tc.tile_pool(name="const", bufs=1) as cpool,
            tc.tile_pool(name="gather", bufs=3) as gpool,
            tc.tile_pool(name="outp", bufs=3) as opool,
        ):
            bx = cpool.tile([P, 5], f32)
            xs_t = cpool.tile([P, NPTS], f32)
            ys_t = cpool.tile([P, NPTS], f32)
            nc.sync.dma_start(out=bx[:], in_=boxes_d[:])
            nc.sync.dma_start(out=xs_t[:], in_=xs_d[:])
            nc.sync.dma_start(out=ys_t[:], in_=ys_d[:])

            cx, cy, w, h, ang = (bx[:, i:i + 1] for i in range(5))

            def t1(name):
                return cpool.tile([P, 1], f32, tag=name, name=name)

            # rad = -ang*pi/180 in (-2pi, 0].  ACT Sin domain is [-pi, pi]:
            #   s_raw = sin(rad + pi)  = -sin(rad)
            #   c_raw = sin(rad + 3pi/2 - 2pi*[arg > pi]) = -cos(rad)
            # signs folded into the b** coefficients below.
            s_arg = t1("s_arg")
            c_arg = t1("c_arg")
            cwrap = t1("cwrap")
            s_raw = t1("s_raw")
            c_raw = t1("c_raw")
            nc.vector.tensor_scalar(out=s_arg, in0=ang, scalar1=-_PI / 180.0,
                                    scalar2=_PI, op0=Alu.mult, op1=Alu.add)
            nc.vector.tensor_scalar(out=s_arg, in0=s_arg, scalar1=-_PI_CLAMP,
                                    scalar2=None, op0=Alu.max)
            nc.vector.tensor_scalar(out=s_arg, in0=s_arg, scalar1=_PI_CLAMP,
                                    scalar2=None, op0=Alu.min)
            nc.scalar.activation(out=s_raw, in_=s_arg, func=Act.Sin)
            nc.vector.tensor_scalar(out=c_arg, in0=ang, scalar1=-_PI / 180.0,
                                    scalar2=1.5 * _PI, op0=Alu.mult, op1=Alu.add)
            nc.vector.tensor_scalar(out=cwrap, in0=c_arg, scalar1=_PI,
                                    scalar2=None, op0=Alu.is_gt)
            nc.vector.scalar_tensor_tensor(out=c_arg, in0=cwrap,
                                           scalar=-_TWO_PI, in1=c_arg,
                                           op0=Alu.mult, op1=Alu.add)
            nc.vector.tensor_scalar(out=c_arg, in0=c_arg, scalar1=-_PI_CLAMP,
                                    scalar2=None, op0=Alu.max)
            nc.vector.tensor_scalar(out=c_arg, in0=c_arg, scalar1=_PI_CLAMP,
                                    scalar2=None, op0=Alu.min)
            nc.scalar.activation(out=c_raw, in_=c_arg, func=Act.Sin)

            # ix = b00*xs + b01*ys + b02 ; iy = b10*xs + b11*ys + b12
            # (pixel coords, align_corners=False; s_raw/c_raw carry -1)
            b00 = t1("b00"); b01 = t1("b01"); b02 = t1("b02")
            b10 = t1("b10"); b11 = t1("b11"); b12 = t1("b12")
            tw = t1("tw"); th = t1("th")
            nc.vector.tensor_scalar(out=tw, in0=w, scalar1=-0.5, scalar2=None,
                                    op0=Alu.mult)
            nc.vector.tensor_scalar(out=th, in0=h, scalar1=-0.5, scalar2=None,
                                    op0=Alu.mult)
            nc.vector.tensor_tensor(out=b00, in0=tw, in1=c_raw, op=Alu.mult)
            nc.vector.tensor_tensor(out=b11, in0=th, in1=c_raw, op=Alu.mult)
            nc.vector.tensor_scalar(out=tw, in0=w, scalar1=-0.5 * H / W,
                                    scalar2=None, op0=Alu.mult)
            nc.vector.tensor_scalar(out=th, in0=h, scalar1=0.5 * W / H,
                                    scalar2=None, op0=Alu.mult)
            nc.vector.tensor_tensor(out=b10, in0=tw, in1=s_raw, op=Alu.mult)
            nc.vector.tensor_tensor(out=b01, in0=th, in1=s_raw, op=Alu.mult)
            nc.vector.tensor_scalar(out=b02, in0=cx, scalar1=-0.5, scalar2=None,
                                    op0=Alu.add)
            nc.vector.tensor_scalar(out=b12, in0=cy, scalar1=-0.5, scalar2=None,
                                    op0=Alu.add)

            def tp(name):
                return cpool.tile([P, NPTS], f32, tag=name, name=name)

            ix = tp("ix"); iy = tp("iy")
            nc.vector.tensor_scalar(out=ix, in0=ys_t, scalar1=b01, scalar2=None,
                                    op0=Alu.mult)
            nc.vector.scalar_tensor_tensor(out=ix, in0=xs_t, scalar=b00,
                                           in1=ix, op0=Alu.mult, op1=Alu.add)
            nc.vector.tensor_scalar(out=ix, in0=ix, scalar1=b02, scalar2=None,
                                    op0=Alu.add)
            nc.vector.tensor_scalar(out=iy, in0=ys_t, scalar1=b11, scalar2=None,
                                    op0=Alu.mult)
            nc.vector.scalar_tensor_tensor(out=iy, in0=xs_t, scalar=b10,
                                           in1=iy, op0=Alu.mult, op1=Alu.add)
            nc.vector.tensor_scalar(out=iy, in0=iy, scalar1=b12, scalar2=None,
                                    op0=Alu.add)

            def magic_floor(out, coord, tmp):
                # exact floor for |coord| < 2^22 via round-to-nearest + fixup
                nc.vector.tensor_scalar(out=out, in0=coord, scalar1=_MAGIC,
                                        scalar2=None, op0=Alu.add)
                nc.vector.tensor_scalar(out=out, in0=out, scalar1=_MAGIC,
                                        scalar2=None, op0=Alu.subtract)
                nc.vector.tensor_tensor(out=tmp, in0=out, in1=coord,
                                        op=Alu.is_gt)
                nc.vector.tensor_tensor(out=out, in0=out, in1=tmp,
                                        op=Alu.subtract)

            def corner_terms(coord, lim, pfx):
                """floor c0, frac fr, u0=(1-fr)*valid(c0), u1=fr*valid(c0+1)"""
                c0 = tp(pfx + "c0")
                c1 = tp(pfx + "c1")
                fr = tp(pfx + "fr")
                u0 = tp(pfx + "u0")
                u1 = tp(pfx + "u1")
                tmp = tp(pfx + "tmp")
                magic_floor(c0, coord, tmp)
                nc.vector.tensor_tensor(out=fr, in0=coord, in1=c0,
                                        op=Alu.subtract)
                nc.vector.tensor_scalar(out=c1, in0=c0, scalar1=1.0,
                                        scalar2=None, op0=Alu.add)
                # valid(c) = [0 <= c <= lim-1] == [c == clip(c, 0, lim-1)]
                nc.vector.tensor_scalar(out=tmp, in0=c0, scalar1=0.0,
                                        scalar2=None, op0=Alu.max)
                nc.vector.tensor_scalar(out=tmp, in0=tmp, scalar1=float(lim - 1),
                                        scalar2=None, op0=Alu.min)
                nc.vector.tensor_tensor(out=u0, in0=c0, in1=tmp, op=Alu.is_equal)
                nc.vector.tensor_scalar(out=tmp, in0=fr, scalar1=-1.0,
                                        scalar2=1.0, op0=Alu.mult, op1=Alu.add)
                nc.vector.tensor_tensor(out=u0, in0=u0, in1=tmp, op=Alu.mult)
                nc.vector.tensor_scalar(out=tmp, in0=c1, scalar1=0.0,
                                        scalar2=None, op0=Alu.max)
                nc.vector.tensor_scalar(out=tmp, in0=tmp, scalar1=float(lim - 1),
                                        scalar2=None, op0=Alu.min)
                nc.vector.tensor_tensor(out=tmp, in0=c1, in1=tmp, op=Alu.is_equal)
                nc.vector.tensor_tensor(out=u1, in0=fr, in1=tmp, op=Alu.mult)
                return c0, c1, u0, u1

            x0f, x1f, ux0, ux1 = corner_terms(ix, W, "x")
            y0f, y1f, uy0, uy1 = corner_terms(iy, H, "y")

            # --- y side: gather element = rows yb, yb+1; yb = clip(y0,0,H-2)
            yb = tp("yb")
            yb1 = tp("yb1")
            tmp = tp("tmp")
            tmp2 = tp("tmp2")
            nc.vector.tensor_scalar(out=yb, in0=y0f, scalar1=0.0,
                                    scalar2=None, op0=Alu.max)
            nc.vector.tensor_scalar(out=yb, in0=yb, scalar1=float(H - 2),
                                    scalar2=None, op0=Alu.min)
            nc.vector.tensor_scalar(out=yb1, in0=yb, scalar1=1.0,
                                    scalar2=None, op0=Alu.add)

            def slot_weight(dst, colt, u_a, c_a, u_b, c_b):
                # dst = u_a*[colt==c_a] + u_b*[colt==c_b]
                nc.vector.tensor_tensor(out=tmp, in0=colt, in1=c_a,
                                        op=Alu.is_equal)
                nc.vector.tensor_tensor(out=dst, in0=u_a, in1=tmp, op=Alu.mult)
                nc.vector.tensor_tensor(out=tmp, in0=colt, in1=c_b,
                                        op=Alu.is_equal)
                nc.vector.tensor_tensor(out=tmp2, in0=u_b, in1=tmp, op=Alu.mult)
                nc.vector.tensor_tensor(out=dst, in0=dst, in1=tmp2, op=Alu.add)

            wy0 = tp("wy0"); wy1 = tp("wy1")
            slot_weight(wy0, yb, uy0, y0f, uy1, y1f)
            slot_weight(wy1, yb1, uy0, y0f, uy1, y1f)

            # --- x side: even col Ecol = x0 + (x0 mod 2), odd col = other
            hx = tp("hx")
            hfl = tp("hfl")
            par = tp("par")
            ecol = tp("ecol")
            ocol = tp("ocol")
            nc.vector.tensor_scalar(out=hx, in0=x0f, scalar1=0.5,
                                    scalar2=None, op0=Alu.mult)
            magic_floor(hfl, hx, tmp)                      # floor(x0/2)
            nc.vector.scalar_tensor_tensor(out=par, in0=hfl, scalar=-2.0,
                                           in1=x0f, op0=Alu.mult, op1=Alu.add)
            nc.vector.tensor_tensor(out=ecol, in0=x0f, in1=par, op=Alu.add)
            nc.vector.tensor_scalar(out=ocol, in0=x0f, scalar1=1.0,
                                    scalar2=None, op0=Alu.add)
            nc.vector.tensor_tensor(out=ocol, in0=ocol, in1=par, op=Alu.subtract)
            nc.vector.tensor_scalar(out=ecol, in0=ecol, scalar1=0.0,
                                    scalar2=None, op0=Alu.max)
            nc.vector.tensor_scalar(out=ecol, in0=ecol, scalar1=float(W - 2),
                                    scalar2=None, op0=Alu.min)
            nc.vector.tensor_scalar(out=ocol, in0=ocol, scalar1=1.0,
                                    scalar2=None, op0=Alu.max)
            nc.vector.tensor_scalar(out=ocol, in0=ocol, scalar1=float(W - 1),
                                    scalar2=None, op0=Alu.min)
            wxe = tp("wxe"); wxo = tp("wxo")
            slot_weight(wxe, ecol, ux0, x0f, ux1, x1f)
            slot_weight(wxo, ocol, ux0, x0f, ux1, x1f)

            # final per-(point, slot) weights
            we0 = tp("we0"); we1 = tp("we1"); wo0 = tp("wo0"); wo1 = tp("wo1")
            nc.vector.tensor_tensor(out=we0, in0=wxe, in1=wy0, op=Alu.mult)
            nc.vector.tensor_tensor(out=we1, in0=wxe, in1=wy1, op=Alu.mult)
            nc.vector.tensor_tensor(out=wo0, in0=wxo, in1=wy0, op=Alu.mult)
            nc.vector.tensor_tensor(out=wo1, in0=wxo, in1=wy1, op=Alu.mult)

            # gather row indices: qE = (Ecol/2)*H + yb = Ecol*(H/2) + yb
            qe = tp("qe"); qo = tp("qo")
            nc.vector.scalar_tensor_tensor(out=qe, in0=ecol, scalar=float(H // 2),
                                           in1=yb, op0=Alu.mult, op1=Alu.add)
            nc.vector.scalar_tensor_tensor(out=qo, in0=ocol, scalar=float(H // 2),
                                           in1=yb, op0=Alu.mult, op1=Alu.add)
            nc.vector.tensor_scalar(out=qo, in0=qo, scalar1=float(-(H // 2)),
                                    scalar2=None, op0=Alu.add)

            qe16 = cpool.tile([P, NPTS], i16, name="qe16")
            qo16 = cpool.tile([P, NPTS], i16, name="qo16")
            nc.vector.tensor_copy(out=qe16[:], in_=qe)
            nc.vector.tensor_copy(out=qo16[:], in_=qo)

            # stage idx to DRAM, reload in the wrapped-16 layout the gather
            # ucode expects: list pos i -> partition i%16 (replicated to all
            # 8 Q7 cores), col i//16; i = point*128 + box.
            nc.sync.dma_start(out=stge[:], in_=qe16[:])
            nc.sync.dma_start(out=stgo[:], in_=qo16[:])
            te = cpool.tile([P, NPTS * 8], i16, name="te")
            to = cpool.tile([P, NPTS * 8], i16, name="to")
            stge_w = stge.ap().rearrange("(b p) j -> p j b", p=16)
            stgo_w = stgo.ap().rearrange("(b p) j -> p j b", p=16)
            for r in range(8):
                nc.sync.dma_start(
                    out=te[16 * r:16 * r + 16, :].rearrange(
                        "p (j b) -> p j b", b=8),
                    in_=stge_w)
                nc.sync.dma_start(
                    out=to[16 * r:16 * r + 16, :].rearrange(
                        "p (j b) -> p j b", b=8),
                    in_=stgo_w)

            # gather + weighted sum, GROUP points per dma_gather call
            for k in range(NPTS // GROUP):
                j0 = k * GROUP
                nidx = GROUP * P
                ge = gpool.tile([P, GROUP * 2 * C], f16, tag="ge", name="ge")
                go = gpool.tile([P, GROUP * 2 * C], f16, tag="go", name="go")
                nc.gpsimd.dma_gather(
                    out_ap=ge[:].rearrange("p (n d) -> p n d", d=2 * C),
                    in_ap=fme_v, idxs_ap=te[:, j0 * 8:(j0 + GROUP) * 8],
                    num_idxs=nidx, num_idxs_reg=nidx, elem_size=2 * C,
                    elem_step=C, single_packet=False, queue_num=0)
                nc.gpsimd.dma_gather(
                    out_ap=go[:].rearrange("p (n d) -> p n d", d=2 * C),
                    in_ap=fmo_v, idxs_ap=to[:, j0 * 8:(j0 + GROUP) * 8],
                    num_idxs=nidx, num_idxs_reg=nidx, elem_size=2 * C,
                    elem_step=C, single_packet=False, queue_num=1)
                ot = opool.tile([P, GROUP * C], f32, tag="ot", name="ot")
                for j in range(GROUP):
                    o = ot[:, j * C:(j + 1) * C]
                    col = j0 + j
                    base = j * 2 * C
                    nc.vector.tensor_scalar(
                        out=o, in0=ge[:, base:base + C],
                        scalar1=we0[:, col:col + 1], scalar2=None, op0=Alu.mult)
                    nc.vector.scalar_tensor_tensor(
                        out=o, in0=ge[:, base + C:base + 2 * C],
                        scalar=we1[:, col:col + 1], in1=o,
                        op0=Alu.mult, op1=Alu.add)
                    nc.vector.scalar_tensor_tensor(
                        out=o, in0=go[:, base:base + C],
                        scalar=wo0[:, col:col + 1], in1=o,
                        op0=Alu.mult, op1=Alu.add)
                    nc.vector.scalar_tensor_tensor(
                        out=o, in0=go[:, base + C:base + 2 * C],
                        scalar=wo1[:, col:col + 1], in1=o,
                        op0=Alu.mult, op1=Alu.add)
                nc.sync.dma_start(out=out_d[:, j0:j0 + GROUP, :], in_=ot[:])

    nc.compile()
    return nc


def _get_program():
    global _compiled
    if _compiled is None:
        _compiled = _build_program()
    return _compiled


def _make_in_maps(feature_map, boxes):
    feature_map = np.ascontiguousarray(feature_map, dtype=np.float32)
    boxes = np.ascontiguousarray(boxes, dtype=np.float32)
    # x-major channels-last, split by x parity:
    # fmT[b, x, y, c]; E rows = (x/2)*H + y for even x
    fmT = feature_map.transpose(0, 3, 2, 1)          # [B, W, H, C]
    fme = np.ascontiguousarray(fmT[:, 0::2]).reshape(B, NROWS, C).astype(np.float16)
    fmo = np.ascontiguousarray(fmT[:, 1::2]).reshape(B, NROWS, C).astype(np.float16)
    # 7x7 affine_grid base coords (align_corners=False), point-major p=ph*7+pw
    xs = ((2.0 * np.arange(OUT_W, dtype=np.float32) + 1.0) / OUT_W - 1.0)
    ys = ((2.0 * np.arange(OUT_H, dtype=np.float32) + 1.0) / OUT_H - 1.0)
    xs_t = np.broadcast_to(np.tile(xs, OUT_H), (P, NPTS)).copy()
    ys_t = np.broadcast_to(np.repeat(ys, OUT_W), (P, NPTS)).copy()

    in_maps = []
    for k in range(N_CORES):
        img = k // (N_CORES // B)
        slot = k % (N_CORES // B)
        in_maps.append({
            "fme": fme[img],
            "fmo": fmo[img],
            "boxes": np.ascontiguousarray(
                boxes[img, slot * P:(slot + 1) * P, :]),
            "xs": xs_t,
            "ys": ys_t,
        })
    return in_maps


def _assemble(results):
    # per-core out: [P, 49, 256] -> full [1024, 256, 7, 7]
    parts = [results[k]["out"] for k in range(N_CORES)]
    full = np.concatenate(parts, axis=0)              # [1024, 49, 256]
    full = full.transpose(0, 2, 1).reshape(B * N, C, OUT_H, OUT_W)
    return np.ascontiguousarray(full)


def run_on_device(feature_map, boxes, trace=False):
    from concourse.bass_utils import run_bass_kernel_spmd

    nc = _get_program()
    in_maps = _make_in_maps(feature_map, boxes)
    res = run_bass_kernel_spmd(nc, in_maps, list(range(N_CORES)), trace=trace)
    return _assemble(res.results), res


def kernel(feature_map, boxes):
    out, _ = run_on_device(feature_map, boxes, trace=False)
    return out
